# revision 36
# baseline (speedup 1.0000x reference)
"""GridRNN Trainium2 kernel (bf16 matmul datapath).

Problem: 2-D grid RNN, B=4, S=T=128, H=256, D=3 depths.
  hx[d][b,i,j] = tanh(xin @ Wx_ih[d].T + bx_ih[d] + hx[d][b,i-1,(j-1)%T] @ Wx_hh[d].T + bx_hh[d])
  hy[d][b,i,j] = tanh(yin @ Wy_ih[d].T + by_ih[d] + hy[d][b,i,j-1]     @ Wy_hh[d].T + by_hh[d])
  (xin/yin = src/trg broadcast at d=0, previous depth's hx/hy for d>0)
  out = stack([hx[D-1], hy[D-1]], axis=-2)   # [B,S,T,2,H]

Key structure: the x-chain and y-chain never mix across depths -> 8 cores =
4 batches x 2 chains.  The x-chain's diagonal dependence hx[i-1,(j-1)%T] is
removed by shearing: u_i[c] = hx[i,(i+c)%T] turns it into a plain carry
u_{i-1}[c], identical in form to the y-chain.  One SPMD program runs on all
8 cores; only the input data (seed, weights) differs per core.  The host
unshears the x outputs and transposes the y outputs.

Numerics: matmul operands (weights + carried state) are bf16 — fp32 matmul
costs 4 cycles/row on the PE vs 1 for bf16.  Accumulation stays fp32 in
PSUM, biases are applied in fp32 by the scalar engine, and tanh rounds to
bf16 on write.  End-to-end L2 rel err vs the fp32 reference ~6e-3 (CPU
emulation), comfortably inside the 2e-2 gate.  Output leaves as bf16 (the
carried state IS bf16, so the host's fp32 upconvert is exact) which halves
output HBM + host traffic.

On-chip layout per step: state u kept as [128(part)=H%128, 2(k=H/128), V=128]
(H on partitions as two k-tiles).  Recurrence out[Hout,V] = W.T tiles (lhsT)
@ state tiles (rhs), accumulated in PSUM, tanh via ScalarE with per-partition
bias.  Depth-0's input term depends only on the step index, so it is
precomputed once as columns and folded into the tanh bias.

This walrus build allows only ONE sync-wait per hardware instruction, so the
kernel is structured to keep Tile's emitted waits at <=1 everywhere: all
constants arrive in two packed DMAs (bf16 + fp32 blobs), absorber ops fold
DMA-queue semaphores into each engine's vector clock, outputs accumulate in
one big SBUF tile and leave in a few large DMAs (few DMA lanes -> short tail
drain).
"""

import numpy as np
import ml_dtypes

import concourse.bass as bass
import concourse.tile as tile
from concourse import mybir
from concourse.bass_utils import run_bass_kernel_spmd

B, S, T, H, D = 4, 128, 128, 256, 3
P = 128          # partitions
K = H // P       # 2 k-tiles of H on partitions
F32 = mybir.dt.float32
BF16 = mybir.dt.bfloat16
TANH = mybir.ActivationFunctionType.Tanh
BF_NP = ml_dtypes.bfloat16

# bf16 blob column layout (bf16 words per partition).  The first SPLIT
# columns (everything depth 0 needs: wih0, seed, identity, whh0) arrive in a
# small leading DMA so the pre0/seedT startup chain and tick 0 run while the
# d1/d2 weights stream in behind them.
W0 = 0                    # wihT d=0: (k, m) -> W0 + k*H + m*P
S0 = W0 + K * H           # seed row (this partition's step row)
I0 = S0 + H               # identity
W1A = I0 + P              # whhT d=0
SPLIT = W1A + K * H
W0B = SPLIT               # wihT d>=1: (d, k, m) -> W0B + ((d-1)*K+k)*H + m*P
W1B = W0B + (D - 1) * K * H  # whhT d>=1
CWH = W1B + (D - 1) * K * H

# fp32 blob column layout
B0 = 0                    # bias cols: (d, m) -> B0 + d*K + m
CWF = B0 + D * K

# output chunk boundaries: big chunks early, tapered at the end so the
# final post-compute DMA is short
OCHUNK_ENDS = [16, 32, 48, 64, 80, 96, 112, 120, 124, 128]

_cache = {}


def _patched_drain_and_barrier(self, tick_clock, wait_clock):
    """Replacement for TileContext._drain_and_barrier.

    This walrus build lowers at most ONE sync-wait per instruction; the stock
    tail drain carries one wait per active proc.  Semantically the waits only
    need to complete before the final barrier's semaphore cleanup, so spread
    them over single-wait NOPs on the sync engine after the drain.
    """
    drain_inst = self.nc.sync.drain()
    wait_clock.add_sem_waits(
        drain_inst.ins, tile.ScopedClock({None: tick_clock.global_clock})
    )
    ins = drain_inst.ins
    si = ins.sync_info
    if si is not None and len(si.on_wait) > 1:
        waits = list(si.on_wait)
        ins.sync_info = mybir.SyncInfo(on_wait=[waits[0]],
                                       on_update=list(si.on_update))
        for w in waits[1:]:
            nop = self.nc.sync.nop(nofuse=True)
            nop.ins.sync_info = mybir.SyncInfo(on_wait=[w], on_update=[])

    self.nc.all_engine_barrier()
    assert self.sems is not None
    popped = self.nc._tile_sem_poison_stack.pop()
    assert popped is self._sem_poison
    self.nc.clear_and_free_semaphores(list(self.sems.allocated().values()))
    self.nc.all_engine_barrier()


tile.TileContext._drain_and_barrier = _patched_drain_and_barrier


_orig_add_instruction = tile.TileContext._add_instruction


def _patched_add_instruction(self, inst):
    """Split multi-sem-wait instructions for the one-wait-per-inst walrus.

    When Tile lowers an instruction with N>1 sem waits, emit N-1 standalone
    EventSemaphore carriers on the same engine immediately before it (the
    engine executes its stream in order, so the waits still gate the
    instruction); the instruction keeps one wait.
    """
    si = inst.sync_info
    if si is not None and len(si.on_wait) > 1:
        waits = list(si.on_wait)
        for i, w in enumerate(waits[:-1]):
            carrier = mybir.InstNoOp(
                name=f"{inst.name}-esw{i}", ins=[], outs=[])
            carrier.engine = inst.engine
            carrier.bass_nofuse = True
            carrier.sync_info = mybir.SyncInfo(on_wait=[w], on_update=[])
            _orig_add_instruction(self, carrier)
        inst.sync_info = mybir.SyncInfo(on_wait=[waits[-1]],
                                        on_update=list(si.on_update))
    _orig_add_instruction(self, inst)


tile.TileContext._add_instruction = _patched_add_instruction


def _build():
    nc = bass.Bass(trn_type="TRN2")

    blobh = nc.dram_tensor("blobh", [P, CWH], BF16, kind="ExternalInput")
    blobf = nc.dram_tensor("blobf", [P, CWF], F32, kind="ExternalInput")
    # DRAM layout mirrors SBUF exactly ([p, s, k, v]) so the output DMA is
    # 128 fully-contiguous runs; the host reassembles H = k*128+p.
    out = nc.dram_tensor("out", [P, S, K, T], BF16, kind="ExternalOutput")
    out_c = out[:, :, :, :]

    with tile.TileContext(nc) as tc:
        with (
            tc.tile_pool(name="consts", bufs=1) as consts,
            tc.tile_pool(name="u0p", bufs=4) as u0p,
            tc.tile_pool(name="u1p", bufs=4) as u1p,
            tc.tile_pool(name="ps0", bufs=3, space="PSUM") as ps0p,
            tc.tile_pool(name="ps1", bufs=3, space="PSUM") as ps1p,
            tc.tile_pool(name="ps2", bufs=2, space="PSUM") as ps2p,
        ):
            # Two separate tiles for the two blob halves: per-tile dependency
            # tracking ties each weight read to the right DMA's semaphore.
            cba = consts.tile([P, SPLIT], BF16)
            nc.gpsimd.dma_start(out=cba, in_=blobh[:, 0:SPLIT])
            cbf = consts.tile([P, CWF], F32)
            nc.gpsimd.dma_start(out=cbf, in_=blobf[:, :])
            cbb = consts.tile([P, CWH - SPLIT], BF16)
            nc.gpsimd.dma_start(out=cbb, in_=blobh[:, SPLIT:])

            def wih(d, k, m):
                if d == 0:
                    c = W0 + k * H + m * P
                    return cba[:, c:c + P]
                c = W0B - SPLIT + ((d - 1) * K + k) * H + m * P
                return cbb[:, c:c + P]

            def whh(d, k, m):
                if d == 0:
                    c = W1A + k * H + m * P
                    return cba[:, c:c + P]
                c = W1B - SPLIT + ((d - 1) * K + k) * H + m * P
                return cbb[:, c:c + P]

            def bias(d, m):
                c = B0 + d * K + m
                return cbf[:, c:c + 1]

            seed_sb = cba[:, S0:S0 + H]
            ident = cba[:, I0:I0 + P]

            zeros = consts.tile([P, K, T], BF16)
            nc.vector.memset(zeros, 0.0)
            # ScalarE absorber: folds the fp32-blob DMA semaphore into ACT's
            # clock
            scr = consts.tile([P, 4], F32)
            nc.scalar.copy(out=scr[:, 0:1], in_=bias(0, 0))

            # ---- seedT[k] = seed[:, k*128:(k+1)*128].T  (PE transpose)
            # The startup PSUM tiles borrow the loop pools' tag slots; their
            # full-region PE writes double as has_written warmups for those
            # banks (all loop groups run DVE-init + start=False).
            seedT_sb = consts.tile([P, K, S], BF16)
            pst = ps1p.tile([P, K, S], BF16, tag="ps1", name="pst")
            for k in range(K):
                nc.tensor.transpose(pst[:, k, :], seed_sb[:, k * P:(k + 1) * P], ident)
            nc.vector.tensor_copy(seedT_sb, pst)

            # ---- pre0[:, m, s] = (W_ih[0] @ seed[s] + bsum[0])[m*128+p]
            pre0_sb = consts.tile([P, K, S], F32)
            psp = ps2p.tile([P, K, S], F32, tag="ps2", name="psp")
            for m in range(K):
                for k in range(K):
                    nc.tensor.matmul(
                        psp[:, m, :], lhsT=wih(0, k, m), rhs=seedT_sb[:, k, :],
                        start=(k == 0), stop=(k == K - 1))
            for m in range(K):
                nc.scalar.activation(
                    pre0_sb[:, m, :], psp[:, m, :],
                    mybir.ActivationFunctionType.Identity, bias=bias(0, m))

            # ---- constant bias planes for d1/d2 (broadcast [P,1] -> [P,K,T])
            biasbc1 = consts.tile([P, K, T], F32)
            biasbc2 = consts.tile([P, K, T], F32)
            for m in range(K):
                nc.vector.tensor_scalar_add(biasbc1[:, m, :], zeros[:, 0, :],
                                            bias(1, m))
                nc.vector.tensor_scalar_add(biasbc2[:, m, :], zeros[:, 0, :],
                                            bias(2, m))

            # ---- main wavefront: tick t runs d0 step t, d1 step t-1, d2 step t-2
            # All d2 outputs accumulate in one big SBUF tile, leaving in a few
            # large SWDGE DMAs (few DMA lanes keeps the tail drain legal).
            #
            # Bias injection runs on DVE so ACT stays at its floor (3 tanh
            # per tick, the serial minimum) and PE at its (20 matmuls): DVE
            # writes the bias plane (d1/d2) or per-step pre0 column (d0) into
            # PSUM, and the matmuls accumulate onto it with start=False --
            # sound because every bank's has_written bits were set by a
            # startup warmup group and no start=True ever clears them again.
            u2all = consts.tile([P, S, K, T], BF16)
            u0, u1 = {}, {}
            u0[-1] = zeros
            u1[-1] = zeros

            def rec_mms(ps, d, u_in, u_prev):
                """Accumulate Wih[d] @ u_in + Whh[d] @ u_prev into ps[:,m,:].

                All matmuls accumulate onto the DVE-initialized PSUM
                (start=False, skip_group_check).  State-independent matmuls
                are emitted before state-dependent ones (ih before hh) so the
                in-order PE stalls as late as possible.
                """
                for m in range(K):
                    if u_in is not None:
                        for k in range(K):
                            nc.tensor.matmul(ps[:, m, :], lhsT=wih(d, k, m),
                                             rhs=u_in[:, k, :],
                                             start=False, stop=False,
                                             skip_group_check=True)
                    for k in range(K):
                        nc.tensor.matmul(ps[:, m, :], lhsT=whh(d, k, m),
                                         rhs=u_prev[:, k, :],
                                         start=False, stop=(k == K - 1),
                                         skip_group_check=True)

            # Warm every remaining psum buffer's has_written bits once with a
            # throwaway start=True group, so all real groups can use the
            # DVE-init + start=False accumulate path uniformly from step 0.
            # (pst/psp above already warmed one slot of ps1/ps2.)
            # ps1/ps2 get a full extra set: pst/psp above leave their slots
            # partially un-warmed (pst is bf16 and half-sized; psp's second
            # m-group opener cleared the first m-group's bits), so their
            # slots are re-warmed by the wrap-around allocation.
            for pool, tag, n in ((ps0p, "ps0", 3), (ps1p, "ps1", 3),
                                 (ps2p, "ps2", 2)):
                for _ in range(n):
                    wtile = pool.tile([P, K, T], F32, tag=tag, name=f"warm_{tag}")
                    nc.tensor.matmul(wtile[:, :, :], lhsT=wih(0, 0, 0),
                                     rhs=zeros[:, :, :], start=True, stop=True)

            for t in range(S + 2):
                if t < S:
                    s = t
                    ps = ps0p.tile([P, K, T], F32, tag="ps0")
                    u = u0p.tile([P, K, T], BF16, tag="u0")
                    for m in range(K):
                        nc.vector.tensor_scalar_add(
                            ps[:, m, :], zeros[:, 0, :],
                            pre0_sb[:, m, s:s + 1])
                    rec_mms(ps, 0, None, u0[s - 1])
                    nc.scalar.activation(u[:, :, :], ps[:, :, :], TANH)
                    u0[s] = u
                if 1 <= t <= S:
                    s = t - 1
                    ps = ps1p.tile([P, K, T], F32, tag="ps1")
                    nc.vector.tensor_copy(ps[:, :, :], biasbc1)
                    rec_mms(ps, 1, u0[s], u1[s - 1])
                    u = u1p.tile([P, K, T], BF16, tag="u1")
                    nc.scalar.activation(u[:, :, :], ps[:, :, :], TANH)
                    u1[s] = u
                if 2 <= t:
                    s = t - 2
                    ps = ps2p.tile([P, K, T], F32, tag="ps2")
                    u2_prev = zeros if s == 0 else u2all[:, s - 1, :, :]
                    nc.vector.tensor_copy(ps[:, :, :], biasbc2)
                    rec_mms(ps, 2, u1[s], u2_prev)
                    nc.scalar.activation(u2all[:, s, :, :], ps[:, :, :], TANH)
                    if (s + 1) in OCHUNK_ENDS:
                        i = OCHUNK_ENDS.index(s + 1)
                        s0 = 0 if i == 0 else OCHUNK_ENDS[i - 1]
                        nc.gpsimd.dma_start(
                            out=out_c[:, s0:s + 1, :, :],
                            in_=u2all[:, s0:s + 1, :, :])
                for dd in (u0, u1):
                    dd.pop(t - 4, None)

    return nc


def _blob_h(seed, wT_ih, wT_hh):
    """Pack per-core bf16 constants into the [P, CWH] blob."""
    b = np.zeros((P, CWH), BF_NP)
    wih_packed = wT_ih.reshape(D, K, P, H).transpose(2, 0, 1, 3).reshape(
        P, D * K * H)
    whh_packed = wT_hh.reshape(D, K, P, H).transpose(2, 0, 1, 3).reshape(
        P, D * K * H)
    b[:, W0:W0 + K * H] = wih_packed[:, :K * H]          # depth 0 (early DMA)
    b[:, W1A:W1A + K * H] = whh_packed[:, :K * H]        # depth 0 (early DMA)
    b[:, W0B:W0B + (D - 1) * K * H] = wih_packed[:, K * H:]
    b[:, W1B:W1B + (D - 1) * K * H] = whh_packed[:, K * H:]
    b[:, S0:S0 + H] = seed
    b[:, I0:I0 + P] = np.eye(P, dtype=np.float32)
    return b


def _blob_f(bs):
    """Pack per-core fp32 bias columns: bsum[d, m*128+p] -> col B0 + d*K + m."""
    b = np.empty((P, CWF), np.float32)
    b[:, B0:B0 + D * K] = bs.reshape(D, K, P).transpose(2, 0, 1).reshape(P, D * K)
    return b


def kernel(src, trg, Wx_ih, Wx_hh, bx_ih, bx_hh, Wy_ih, Wy_hh, by_ih, by_hh):
    if "nc" not in _cache:
        _cache["nc"] = _build()
    nc = _cache["nc"]

    def tr(w):  # [D,H,H] -> W[d].T contiguous
        return np.ascontiguousarray(np.swapaxes(np.asarray(w, np.float32), 1, 2))

    src = np.asarray(src, np.float32)
    trg = np.asarray(trg, np.float32)
    wx_ihT, wx_hhT = tr(Wx_ih), tr(Wx_hh)
    wy_ihT, wy_hhT = tr(Wy_ih), tr(Wy_hh)
    bx = np.asarray(bx_ih, np.float32) + np.asarray(bx_hh, np.float32)
    by = np.asarray(by_ih, np.float32) + np.asarray(by_hh, np.float32)

    in_maps = []
    for b in range(B):  # cores 0-3: x chains
        in_maps.append({"blobh": _blob_h(src[b], wx_ihT, wx_hhT),
                        "blobf": _blob_f(bx)})
    for b in range(B):  # cores 4-7: y chains
        in_maps.append({"blobh": _blob_h(trg[b], wy_ihT, wy_hhT),
                        "blobf": _blob_f(by)})

    _cache["last_in_maps"] = in_maps
    globals()["_last_in_maps"] = in_maps
    res = run_bass_kernel_spmd(nc, in_maps, list(range(8)))

    out = np.empty((B, S, T, 2, H), np.float32)
    ii = np.arange(S)[:, None]
    jj = np.arange(T)[None, :]
    idx = (jj - ii) % T  # hx[i,j] = u_i[(j-i)%T]
    for b in range(B):
        # raw core output [p, s, k, v] -> [s, H=k*128+p, v]
        raw = np.asarray(res.results[b]["out"]).astype(np.float32)
        arr = raw.transpose(1, 2, 0, 3).reshape(S, H, T)
        hx = np.take_along_axis(arr, idx[:, None, :], axis=2)  # [s, H, j]
        out[b, :, :, 0, :] = hx.transpose(0, 2, 1)
        raw = np.asarray(res.results[B + b]["out"]).astype(np.float32)
        arr = raw.transpose(1, 2, 0, 3).reshape(S, H, T)
        out[b, :, :, 1, :] = arr.transpose(2, 0, 1)  # [j, H, i] -> [i, j, H]
    return out


# revision 38
# speedup vs baseline: 1.1826x; 1.1826x over previous
"""GridRNN Trainium2 kernel (bf16 matmul datapath).

Problem: 2-D grid RNN, B=4, S=T=128, H=256, D=3 depths.
  hx[d][b,i,j] = tanh(xin @ Wx_ih[d].T + bx_ih[d] + hx[d][b,i-1,(j-1)%T] @ Wx_hh[d].T + bx_hh[d])
  hy[d][b,i,j] = tanh(yin @ Wy_ih[d].T + by_ih[d] + hy[d][b,i,j-1]     @ Wy_hh[d].T + by_hh[d])
  (xin/yin = src/trg broadcast at d=0, previous depth's hx/hy for d>0)
  out = stack([hx[D-1], hy[D-1]], axis=-2)   # [B,S,T,2,H]

Key structure: the x-chain and y-chain never mix across depths -> 8 cores =
4 batches x 2 chains.  The x-chain's diagonal dependence hx[i-1,(j-1)%T] is
removed by shearing: u_i[c] = hx[i,(i+c)%T] turns it into a plain carry
u_{i-1}[c], identical in form to the y-chain.  One SPMD program runs on all
8 cores; only the input data (seed, weights) differs per core.  The host
unshears the x outputs and transposes the y outputs.

Numerics: matmul operands (weights + carried state) are bf16 — fp32 matmul
costs 4 cycles/row on the PE vs 1 for bf16.  Accumulation stays fp32 in
PSUM, biases are applied in fp32 by the scalar engine, and tanh rounds to
bf16 on write.  End-to-end L2 rel err vs the fp32 reference ~6e-3 (CPU
emulation), comfortably inside the 2e-2 gate.  Output leaves as bf16 (the
carried state IS bf16, so the host's fp32 upconvert is exact) which halves
output HBM + host traffic.

On-chip layout per step: state u kept as [128(part)=H%128, 2(k=H/128), V=128]
(H on partitions as two k-tiles).  Recurrence out[Hout,V] = W.T tiles (lhsT)
@ state tiles (rhs), accumulated in PSUM, tanh via ScalarE with per-partition
bias.  Depth-0's input term depends only on the step index, so it is
precomputed once as columns and folded into the tanh bias.

This walrus build allows only ONE sync-wait per hardware instruction, so the
kernel is structured to keep Tile's emitted waits at <=1 everywhere: all
constants arrive in two packed DMAs (bf16 + fp32 blobs), absorber ops fold
DMA-queue semaphores into each engine's vector clock, outputs accumulate in
one big SBUF tile and leave in a few large DMAs (few DMA lanes -> short tail
drain).
"""

import numpy as np
import ml_dtypes

import concourse.bass as bass
import concourse.tile as tile
from concourse import mybir
from concourse.bass_utils import run_bass_kernel_spmd

B, S, T, H, D = 4, 128, 128, 256, 3
P = 128          # partitions
K = H // P       # 2 k-tiles of H on partitions
F32 = mybir.dt.float32
BF16 = mybir.dt.bfloat16
TANH = mybir.ActivationFunctionType.Tanh
BF_NP = ml_dtypes.bfloat16

# bf16 blob column layout (bf16 words per partition).  The first SPLIT
# columns (everything depth 0 needs: wih0, seed, identity, whh0) arrive in a
# small leading DMA so the pre0/seedT startup chain and tick 0 run while the
# d1/d2 weights stream in behind them.
W0 = 0                    # wihT d=0: (k, m) -> W0 + k*H + m*P
S0 = W0 + K * H           # seed row (this partition's step row)
I0 = S0 + H               # identity
W1A = I0 + P              # whhT d=0
SPLIT = W1A + K * H
W0B = SPLIT               # wihT d>=1: (d, k, m) -> W0B + ((d-1)*K+k)*H + m*P
W1B = W0B + (D - 1) * K * H  # whhT d>=1
CWH = W1B + (D - 1) * K * H

# fp32 blob column layout
B0 = 0                    # bias cols: (d, m) -> B0 + d*K + m
CWF = B0 + D * K

# output chunk boundaries: big chunks early, tapered at the end so the
# final post-compute DMA is short
OCHUNK_ENDS = [16, 32, 48, 64, 80, 96, 112, 120, 124, 128]

_cache = {}


def _patched_drain_and_barrier(self, tick_clock, wait_clock):
    """Replacement for TileContext._drain_and_barrier.

    This walrus build lowers at most ONE sync-wait per instruction; the stock
    tail drain carries one wait per active proc.  Semantically the waits only
    need to complete before the final barrier's semaphore cleanup, so spread
    them over single-wait NOPs on the sync engine after the drain.
    """
    drain_inst = self.nc.sync.drain()
    wait_clock.add_sem_waits(
        drain_inst.ins, tile.ScopedClock({None: tick_clock.global_clock})
    )
    ins = drain_inst.ins
    si = ins.sync_info
    if si is not None and len(si.on_wait) > 1:
        waits = list(si.on_wait)
        ins.sync_info = mybir.SyncInfo(on_wait=[waits[0]],
                                       on_update=list(si.on_update))
        for w in waits[1:]:
            nop = self.nc.sync.nop(nofuse=True)
            nop.ins.sync_info = mybir.SyncInfo(on_wait=[w], on_update=[])

    self.nc.all_engine_barrier()
    assert self.sems is not None
    popped = self.nc._tile_sem_poison_stack.pop()
    assert popped is self._sem_poison
    self.nc.clear_and_free_semaphores(list(self.sems.allocated().values()))
    self.nc.all_engine_barrier()


tile.TileContext._drain_and_barrier = _patched_drain_and_barrier


_orig_add_instruction = tile.TileContext._add_instruction


def _patched_add_instruction(self, inst):
    """Split multi-sem-wait instructions for the one-wait-per-inst walrus.

    When Tile lowers an instruction with N>1 sem waits, emit N-1 standalone
    EventSemaphore carriers on the same engine immediately before it (the
    engine executes its stream in order, so the waits still gate the
    instruction); the instruction keeps one wait.
    """
    si = inst.sync_info
    if si is not None and len(si.on_wait) > 1:
        waits = list(si.on_wait)
        for i, w in enumerate(waits[:-1]):
            carrier = mybir.InstNoOp(
                name=f"{inst.name}-esw{i}", ins=[], outs=[])
            carrier.engine = inst.engine
            carrier.bass_nofuse = True
            carrier.sync_info = mybir.SyncInfo(on_wait=[w], on_update=[])
            _orig_add_instruction(self, carrier)
        inst.sync_info = mybir.SyncInfo(on_wait=[waits[-1]],
                                        on_update=list(si.on_update))
    _orig_add_instruction(self, inst)


tile.TileContext._add_instruction = _patched_add_instruction


def _build():
    nc = bass.Bass(trn_type="TRN2")

    blobh = nc.dram_tensor("blobh", [P, CWH], BF16, kind="ExternalInput")
    blobf = nc.dram_tensor("blobf", [P, CWF], F32, kind="ExternalInput")
    # DRAM layout mirrors SBUF exactly ([p, s, k, v]) so the output DMA is
    # 128 fully-contiguous runs; the host reassembles H = k*128+p.
    out = nc.dram_tensor("out", [P, S, K, T], BF16, kind="ExternalOutput")
    out_c = out[:, :, :, :]

    with tile.TileContext(nc) as tc:
        with (
            tc.tile_pool(name="consts", bufs=1) as consts,
            tc.tile_pool(name="u0p", bufs=4) as u0p,
            tc.tile_pool(name="u1p", bufs=4) as u1p,
            tc.tile_pool(name="ps0", bufs=2, space="PSUM") as ps0p,
            tc.tile_pool(name="ps1", bufs=2, space="PSUM") as ps1p,
            tc.tile_pool(name="ps2", bufs=2, space="PSUM") as ps2p,
            tc.tile_pool(name="psi", bufs=1, space="PSUM") as psip,
        ):
            # Two separate tiles for the two blob halves: per-tile dependency
            # tracking ties each weight read to the right DMA's semaphore.
            cba = consts.tile([P, SPLIT], BF16)
            nc.gpsimd.dma_start(out=cba, in_=blobh[:, 0:SPLIT])
            cbf = consts.tile([P, CWF], F32)
            nc.gpsimd.dma_start(out=cbf, in_=blobf[:, :])
            cbb = consts.tile([P, CWH - SPLIT], BF16)
            nc.gpsimd.dma_start(out=cbb, in_=blobh[:, SPLIT:])

            def wih(d, k, m):
                if d == 0:
                    c = W0 + k * H + m * P
                    return cba[:, c:c + P]
                c = W0B - SPLIT + ((d - 1) * K + k) * H + m * P
                return cbb[:, c:c + P]

            def whh(d, k, m):
                if d == 0:
                    c = W1A + k * H + m * P
                    return cba[:, c:c + P]
                c = W1B - SPLIT + ((d - 1) * K + k) * H + m * P
                return cbb[:, c:c + P]

            def bias(d, m):
                c = B0 + d * K + m
                return cbf[:, c:c + 1]

            seed_sb = cba[:, S0:S0 + H]
            ident = cba[:, I0:I0 + P]

            zeros = consts.tile([P, K, T], BF16)
            nc.vector.memset(zeros, 0.0)
            # ScalarE absorber: folds the fp32-blob DMA semaphore into ACT's
            # clock
            scr = consts.tile([P, 4], F32)
            nc.scalar.copy(out=scr[:, 0:1], in_=bias(0, 0))

            # ---- seedT[k] = seed[:, k*128:(k+1)*128].T  (PE transpose)
            # The startup PSUM tiles borrow the loop pools' tag slots; their
            # full-region PE writes double as has_written warmups for those
            # banks (all loop groups run DVE-init + start=False).
            seedT_sb = consts.tile([P, K, S], BF16)
            pst = psip.tile([P, K, S], BF16, tag="init", name="pst")
            for k in range(K):
                nc.tensor.transpose(pst[:, k, :], seed_sb[:, k * P:(k + 1) * P], ident)
            nc.vector.tensor_copy(seedT_sb, pst)

            # ---- pre0[:, m, s] = (W_ih[0] @ seed[s] + bsum[0])[m*128+p]
            pre0_sb = consts.tile([P, K, S], F32)
            psp = psip.tile([P, K, S], F32, tag="init2", name="psp")
            for m in range(K):
                for k in range(K):
                    nc.tensor.matmul(
                        psp[:, m, :], lhsT=wih(0, k, m), rhs=seedT_sb[:, k, :],
                        start=(k == 0), stop=(k == K - 1))
            for m in range(K):
                nc.scalar.activation(
                    pre0_sb[:, m, :], psp[:, m, :],
                    mybir.ActivationFunctionType.Identity, bias=bias(0, m))

            # ---- constant bias planes for d1/d2 (broadcast [P,1] -> [P,K,T])
            biasbc1 = consts.tile([P, K, T], F32)
            biasbc2 = consts.tile([P, K, T], F32)
            for m in range(K):
                nc.vector.tensor_scalar_add(biasbc1[:, m, :], zeros[:, 0, :],
                                            bias(1, m))
                nc.vector.tensor_scalar_add(biasbc2[:, m, :], zeros[:, 0, :],
                                            bias(2, m))

            # ---- main wavefront: tick t runs d0 step t, d1 step t-1, d2 step t-2
            # All d2 outputs accumulate in one big SBUF tile, leaving in a few
            # large SWDGE DMAs (few DMA lanes keeps the tail drain legal).
            #
            # Bias injection runs on DVE so ACT stays at its floor (3 tanh
            # per tick, the serial minimum) and PE at its (20 matmuls): DVE
            # writes the bias plane (d1/d2) or per-step pre0 column (d0) into
            # PSUM, and the matmuls accumulate onto it with start=False --
            # sound because every bank's has_written bits were set by a
            # startup warmup group and no start=True ever clears them again.
            u2all = consts.tile([P, S, K, T], BF16)
            u0, u1 = {}, {}
            u0[-1] = zeros
            u1[-1] = zeros

            def rec_mms(ps, d, u_in, u_prev):
                """Accumulate Wih[d] @ u_in + Whh[d] @ u_prev into ps[:,m,:].

                All matmuls accumulate onto the DVE-initialized PSUM
                (start=False, skip_group_check).  State-independent matmuls
                are emitted before state-dependent ones (ih before hh) so the
                in-order PE stalls as late as possible.
                """
                for m in range(K):
                    if u_in is not None:
                        for k in range(K):
                            nc.tensor.matmul(ps[:, m, :], lhsT=wih(d, k, m),
                                             rhs=u_in[:, k, :],
                                             start=False, stop=False,
                                             skip_group_check=True)
                    for k in range(K):
                        nc.tensor.matmul(ps[:, m, :], lhsT=whh(d, k, m),
                                         rhs=u_prev[:, k, :],
                                         start=False, stop=(k == K - 1),
                                         skip_group_check=True)

            # Warm every remaining psum buffer's has_written bits once with a
            # throwaway start=True group, so all real groups can use the
            # DVE-init + start=False accumulate path uniformly from step 0.
            # (pst/psp above already warmed one slot of ps1/ps2.)
            for pool, tag, n in ((ps0p, "ps0", 2), (ps1p, "ps1", 2),
                                 (ps2p, "ps2", 2)):
                for _ in range(n):
                    wtile = pool.tile([P, K, T], F32, tag=tag, name=f"warm_{tag}")
                    nc.tensor.matmul(wtile[:, :, :], lhsT=wih(0, 0, 0),
                                     rhs=zeros[:, :, :], start=True, stop=True)

            for t in range(S + 2):
                if t < S:
                    s = t
                    ps = ps0p.tile([P, K, T], F32, tag="ps0")
                    u = u0p.tile([P, K, T], BF16, tag="u0")
                    # pre0 column s broadcast over T via a 0-stride AP
                    src_ap, dst_ap = bass.broadcast_tensor_aps(
                        pre0_sb[:, :, s:s + 1], ps[:, :, :])
                    nc.vector.tensor_copy(dst_ap, src_ap)
                    rec_mms(ps, 0, None, u0[s - 1])
                    nc.scalar.activation(u[:, :, :], ps[:, :, :], TANH)
                    u0[s] = u
                if 1 <= t <= S:
                    s = t - 1
                    ps = ps1p.tile([P, K, T], F32, tag="ps1")
                    nc.vector.tensor_copy(ps[:, :, :], biasbc1)
                    rec_mms(ps, 1, u0[s], u1[s - 1])
                    u = u1p.tile([P, K, T], BF16, tag="u1")
                    nc.scalar.activation(u[:, :, :], ps[:, :, :], TANH)
                    u1[s] = u
                if 2 <= t:
                    s = t - 2
                    ps = ps2p.tile([P, K, T], F32, tag="ps2")
                    u2_prev = zeros if s == 0 else u2all[:, s - 1, :, :]
                    nc.vector.tensor_copy(ps[:, :, :], biasbc2)
                    rec_mms(ps, 2, u1[s], u2_prev)
                    nc.scalar.activation(u2all[:, s, :, :], ps[:, :, :], TANH)
                    if (s + 1) in OCHUNK_ENDS:
                        i = OCHUNK_ENDS.index(s + 1)
                        s0 = 0 if i == 0 else OCHUNK_ENDS[i - 1]
                        nc.gpsimd.dma_start(
                            out=out_c[:, s0:s + 1, :, :],
                            in_=u2all[:, s0:s + 1, :, :])
                for dd in (u0, u1):
                    dd.pop(t - 4, None)

    return nc


def _blob_h(seed, wT_ih, wT_hh):
    """Pack per-core bf16 constants into the [P, CWH] blob."""
    b = np.zeros((P, CWH), BF_NP)
    wih_packed = wT_ih.reshape(D, K, P, H).transpose(2, 0, 1, 3).reshape(
        P, D * K * H)
    whh_packed = wT_hh.reshape(D, K, P, H).transpose(2, 0, 1, 3).reshape(
        P, D * K * H)
    b[:, W0:W0 + K * H] = wih_packed[:, :K * H]          # depth 0 (early DMA)
    b[:, W1A:W1A + K * H] = whh_packed[:, :K * H]        # depth 0 (early DMA)
    b[:, W0B:W0B + (D - 1) * K * H] = wih_packed[:, K * H:]
    b[:, W1B:W1B + (D - 1) * K * H] = whh_packed[:, K * H:]
    b[:, S0:S0 + H] = seed
    b[:, I0:I0 + P] = np.eye(P, dtype=np.float32)
    return b


def _blob_f(bs):
    """Pack per-core fp32 bias columns: bsum[d, m*128+p] -> col B0 + d*K + m."""
    b = np.empty((P, CWF), np.float32)
    b[:, B0:B0 + D * K] = bs.reshape(D, K, P).transpose(2, 0, 1).reshape(P, D * K)
    return b


def kernel(src, trg, Wx_ih, Wx_hh, bx_ih, bx_hh, Wy_ih, Wy_hh, by_ih, by_hh):
    if "nc" not in _cache:
        _cache["nc"] = _build()
    nc = _cache["nc"]

    def tr(w):  # [D,H,H] -> W[d].T contiguous
        return np.ascontiguousarray(np.swapaxes(np.asarray(w, np.float32), 1, 2))

    src = np.asarray(src, np.float32)
    trg = np.asarray(trg, np.float32)
    wx_ihT, wx_hhT = tr(Wx_ih), tr(Wx_hh)
    wy_ihT, wy_hhT = tr(Wy_ih), tr(Wy_hh)
    bx = np.asarray(bx_ih, np.float32) + np.asarray(bx_hh, np.float32)
    by = np.asarray(by_ih, np.float32) + np.asarray(by_hh, np.float32)

    in_maps = []
    for b in range(B):  # cores 0-3: x chains
        in_maps.append({"blobh": _blob_h(src[b], wx_ihT, wx_hhT),
                        "blobf": _blob_f(bx)})
    for b in range(B):  # cores 4-7: y chains
        in_maps.append({"blobh": _blob_h(trg[b], wy_ihT, wy_hhT),
                        "blobf": _blob_f(by)})

    _cache["last_in_maps"] = in_maps
    globals()["_last_in_maps"] = in_maps
    res = run_bass_kernel_spmd(nc, in_maps, list(range(8)))

    out = np.empty((B, S, T, 2, H), np.float32)
    ii = np.arange(S)[:, None]
    jj = np.arange(T)[None, :]
    idx = (jj - ii) % T  # hx[i,j] = u_i[(j-i)%T]
    for b in range(B):
        # raw core output [p, s, k, v] -> [s, H=k*128+p, v]
        raw = np.asarray(res.results[b]["out"]).astype(np.float32)
        arr = raw.transpose(1, 2, 0, 3).reshape(S, H, T)
        hx = np.take_along_axis(arr, idx[:, None, :], axis=2)  # [s, H, j]
        out[b, :, :, 0, :] = hx.transpose(0, 2, 1)
        raw = np.asarray(res.results[B + b]["out"]).astype(np.float32)
        arr = raw.transpose(1, 2, 0, 3).reshape(S, H, T)
        out[b, :, :, 1, :] = arr.transpose(2, 0, 1)  # [j, H, i] -> [i, j, H]
    return out


# revision 41
# speedup vs baseline: 1.1960x; 1.0113x over previous
"""GridRNN Trainium2 kernel (bf16 matmul datapath).

Problem: 2-D grid RNN, B=4, S=T=128, H=256, D=3 depths.
  hx[d][b,i,j] = tanh(xin @ Wx_ih[d].T + bx_ih[d] + hx[d][b,i-1,(j-1)%T] @ Wx_hh[d].T + bx_hh[d])
  hy[d][b,i,j] = tanh(yin @ Wy_ih[d].T + by_ih[d] + hy[d][b,i,j-1]     @ Wy_hh[d].T + by_hh[d])
  (xin/yin = src/trg broadcast at d=0, previous depth's hx/hy for d>0)
  out = stack([hx[D-1], hy[D-1]], axis=-2)   # [B,S,T,2,H]

Key structure: the x-chain and y-chain never mix across depths -> 8 cores =
4 batches x 2 chains.  The x-chain's diagonal dependence hx[i-1,(j-1)%T] is
removed by shearing: u_i[c] = hx[i,(i+c)%T] turns it into a plain carry
u_{i-1}[c], identical in form to the y-chain.  One SPMD program runs on all
8 cores; only the input data (seed, weights) differs per core.  The host
unshears the x outputs and transposes the y outputs.

Numerics: matmul operands (weights + carried state) are bf16 — fp32 matmul
costs 4 cycles/row on the PE vs 1 for bf16.  Accumulation stays fp32 in
PSUM, biases are applied in fp32 by the scalar engine, and tanh rounds to
bf16 on write.  End-to-end L2 rel err vs the fp32 reference ~6e-3 (CPU
emulation), comfortably inside the 2e-2 gate.  Output leaves as bf16 (the
carried state IS bf16, so the host's fp32 upconvert is exact) which halves
output HBM + host traffic.

On-chip layout per step: state u kept as [128(part)=H%128, 2(k=H/128), V=128]
(H on partitions as two k-tiles).  Recurrence out[Hout,V] = W.T tiles (lhsT)
@ state tiles (rhs), accumulated in PSUM, tanh via ScalarE with per-partition
bias.  Depth-0's input term depends only on the step index, so it is
precomputed once as columns and folded into the tanh bias.

This walrus build allows only ONE sync-wait per hardware instruction, so the
kernel is structured to keep Tile's emitted waits at <=1 everywhere: all
constants arrive in two packed DMAs (bf16 + fp32 blobs), absorber ops fold
DMA-queue semaphores into each engine's vector clock, outputs accumulate in
one big SBUF tile and leave in a few large DMAs (few DMA lanes -> short tail
drain).
"""

import numpy as np
import ml_dtypes

import concourse.bass as bass
import concourse.tile as tile
from concourse import mybir
from concourse.bass_utils import run_bass_kernel_spmd

B, S, T, H, D = 4, 128, 128, 256, 3
P = 128          # partitions
K = H // P       # 2 k-tiles of H on partitions
F32 = mybir.dt.float32
BF16 = mybir.dt.bfloat16
TANH = mybir.ActivationFunctionType.Tanh
BF_NP = ml_dtypes.bfloat16

# bf16 blob column layout (bf16 words per partition).  The first SPLIT
# columns (everything depth 0 needs: wih0, seed, identity, whh0) arrive in a
# small leading DMA so the pre0/seedT startup chain and tick 0 run while the
# d1/d2 weights stream in behind them.
W0 = 0                    # wihT d=0: (k, m) -> W0 + k*H + m*P
S0 = W0 + K * H           # seed row (this partition's step row)
I0 = S0 + H               # identity
W1A = I0 + P              # whhT d=0
SPLIT = W1A + K * H
W0B = SPLIT               # wihT d>=1: (d, k, m) -> W0B + ((d-1)*K+k)*H + m*P
W1B = W0B + (D - 1) * K * H  # whhT d>=1
CWH = W1B + (D - 1) * K * H

# fp32 blob column layout
B0 = 0                    # bias cols: (d, m) -> B0 + d*K + m
CWF = B0 + D * K

# output chunk boundaries: big chunks early, tapered at the end so the
# final post-compute DMA is short
OCHUNK_ENDS = [16, 32, 48, 64, 80, 96, 112, 120, 124, 128]

_cache = {}


def _patched_drain_and_barrier(self, tick_clock, wait_clock):
    """Replacement for TileContext._drain_and_barrier.

    This walrus build lowers at most ONE sync-wait per instruction; the stock
    tail drain carries one wait per active proc.  Semantically the waits only
    need to complete before the final barrier's semaphore cleanup, so spread
    them over single-wait NOPs on the sync engine after the drain.
    """
    drain_inst = self.nc.sync.drain()
    wait_clock.add_sem_waits(
        drain_inst.ins, tile.ScopedClock({None: tick_clock.global_clock})
    )
    ins = drain_inst.ins
    si = ins.sync_info
    if si is not None and len(si.on_wait) > 1:
        waits = list(si.on_wait)
        ins.sync_info = mybir.SyncInfo(on_wait=[waits[0]],
                                       on_update=list(si.on_update))
        for w in waits[1:]:
            nop = self.nc.sync.nop(nofuse=True)
            nop.ins.sync_info = mybir.SyncInfo(on_wait=[w], on_update=[])

    self.nc.all_engine_barrier()
    assert self.sems is not None
    popped = self.nc._tile_sem_poison_stack.pop()
    assert popped is self._sem_poison
    self.nc.clear_and_free_semaphores(list(self.sems.allocated().values()))
    self.nc.all_engine_barrier()


tile.TileContext._drain_and_barrier = _patched_drain_and_barrier


_orig_add_instruction = tile.TileContext._add_instruction


def _patched_add_instruction(self, inst):
    """Split multi-sem-wait instructions for the one-wait-per-inst walrus.

    When Tile lowers an instruction with N>1 sem waits, emit N-1 standalone
    EventSemaphore carriers on the same engine immediately before it (the
    engine executes its stream in order, so the waits still gate the
    instruction); the instruction keeps one wait.
    """
    si = inst.sync_info
    if si is not None and len(si.on_wait) > 1:
        waits = list(si.on_wait)
        for i, w in enumerate(waits[:-1]):
            carrier = mybir.InstNoOp(
                name=f"{inst.name}-esw{i}", ins=[], outs=[])
            carrier.engine = inst.engine
            carrier.bass_nofuse = True
            carrier.sync_info = mybir.SyncInfo(on_wait=[w], on_update=[])
            _orig_add_instruction(self, carrier)
        inst.sync_info = mybir.SyncInfo(on_wait=[waits[-1]],
                                        on_update=list(si.on_update))
    _orig_add_instruction(self, inst)


tile.TileContext._add_instruction = _patched_add_instruction


def _build():
    nc = bass.Bass(trn_type="TRN2")

    blobh = nc.dram_tensor("blobh", [P, CWH], BF16, kind="ExternalInput")
    blobf = nc.dram_tensor("blobf", [P, CWF], F32, kind="ExternalInput")
    # DRAM layout mirrors SBUF exactly ([p, s, k, v]) so the output DMA is
    # 128 fully-contiguous runs; the host reassembles H = k*128+p.
    out = nc.dram_tensor("out", [P, S, K, T], BF16, kind="ExternalOutput")
    out_c = out[:, :, :, :]

    with tile.TileContext(nc) as tc:
        with (
            tc.tile_pool(name="consts", bufs=1) as consts,
            tc.tile_pool(name="u0p", bufs=4) as u0p,
            tc.tile_pool(name="u1p", bufs=4) as u1p,
            tc.tile_pool(name="ps0", bufs=3, space="PSUM") as ps0p,
            tc.tile_pool(name="ps1", bufs=2, space="PSUM") as ps1p,
            tc.tile_pool(name="ps2", bufs=2, space="PSUM") as ps2p,
            tc.tile_pool(name="psi", bufs=1, space="PSUM") as psip,
        ):
            # Two separate tiles for the two blob halves: per-tile dependency
            # tracking ties each weight read to the right DMA's semaphore.
            cba = consts.tile([P, SPLIT], BF16)
            nc.gpsimd.dma_start(out=cba, in_=blobh[:, 0:SPLIT])
            cbf = consts.tile([P, CWF], F32)
            nc.gpsimd.dma_start(out=cbf, in_=blobf[:, :])
            cbb = consts.tile([P, CWH - SPLIT], BF16)
            nc.gpsimd.dma_start(out=cbb, in_=blobh[:, SPLIT:])

            def wih(d, k, m):
                if d == 0:
                    c = W0 + k * H + m * P
                    return cba[:, c:c + P]
                c = W0B - SPLIT + ((d - 1) * K + k) * H + m * P
                return cbb[:, c:c + P]

            def whh(d, k, m):
                if d == 0:
                    c = W1A + k * H + m * P
                    return cba[:, c:c + P]
                c = W1B - SPLIT + ((d - 1) * K + k) * H + m * P
                return cbb[:, c:c + P]

            def bias(d, m):
                c = B0 + d * K + m
                return cbf[:, c:c + 1]

            seed_sb = cba[:, S0:S0 + H]
            ident = cba[:, I0:I0 + P]

            zeros = consts.tile([P, K, T], BF16)
            nc.vector.memset(zeros, 0.0)
            # ScalarE absorber: folds the fp32-blob DMA semaphore into ACT's
            # clock
            scr = consts.tile([P, 4], F32)
            nc.scalar.copy(out=scr[:, 0:1], in_=bias(0, 0))

            # ---- seedT[k] = seed[:, k*128:(k+1)*128].T  (PE transpose)
            # The startup PSUM tiles borrow the loop pools' tag slots; their
            # full-region PE writes double as has_written warmups for those
            # banks (all loop groups run DVE-init + start=False).
            seedT_sb = consts.tile([P, K, S], BF16)
            pst = psip.tile([P, K, S], BF16, tag="init", name="pst")
            for k in range(K):
                nc.tensor.transpose(pst[:, k, :], seed_sb[:, k * P:(k + 1) * P], ident)
            nc.vector.tensor_copy(seedT_sb, pst)

            # ---- pre0[:, m, s] = (W_ih[0] @ seed[s] + bsum[0])[m*128+p]
            pre0_sb = consts.tile([P, K, S], F32)
            psp = psip.tile([P, K, S], F32, tag="init", name="psp")
            for m in range(K):
                for k in range(K):
                    nc.tensor.matmul(
                        psp[:, m, :], lhsT=wih(0, k, m), rhs=seedT_sb[:, k, :],
                        start=(k == 0), stop=(k == K - 1))
            for m in range(K):
                nc.scalar.activation(
                    pre0_sb[:, m, :], psp[:, m, :],
                    mybir.ActivationFunctionType.Identity, bias=bias(0, m))

            # ---- constant bias planes for d1/d2 (broadcast [P,1] -> [P,K,T])
            biasbc1 = consts.tile([P, K, T], F32)
            biasbc2 = consts.tile([P, K, T], F32)
            for m in range(K):
                nc.vector.tensor_scalar_add(biasbc1[:, m, :], zeros[:, 0, :],
                                            bias(1, m))
                nc.vector.tensor_scalar_add(biasbc2[:, m, :], zeros[:, 0, :],
                                            bias(2, m))

            # ---- main wavefront: tick t runs d0 step t, d1 step t-1, d2 step t-2
            # All d2 outputs accumulate in one big SBUF tile, leaving in a few
            # large SWDGE DMAs (few DMA lanes keeps the tail drain legal).
            #
            # Bias injection runs on DVE so ACT stays at its floor (3 tanh
            # per tick, the serial minimum) and PE at its (20 matmuls): DVE
            # writes the bias plane (d1/d2) or per-step pre0 column (d0) into
            # PSUM, and the matmuls accumulate onto it with start=False --
            # sound because every bank's has_written bits were set by a
            # startup warmup group and no start=True ever clears them again.
            u2all = consts.tile([P, S, K, T], BF16)
            u0, u1 = {}, {}
            u0[-1] = zeros
            u1[-1] = zeros

            def rec_mms(ps, d, u_in, u_prev):
                """Accumulate Wih[d] @ u_in + Whh[d] @ u_prev into ps[:,m,:].

                All matmuls accumulate onto the DVE-initialized PSUM
                (start=False, skip_group_check).  State-independent matmuls
                are emitted before state-dependent ones (ih before hh) so the
                in-order PE stalls as late as possible.
                """
                for m in range(K):
                    if u_in is not None:
                        for k in range(K):
                            nc.tensor.matmul(ps[:, m, :], lhsT=wih(d, k, m),
                                             rhs=u_in[:, k, :],
                                             start=False, stop=False,
                                             skip_group_check=True)
                    for k in range(K):
                        nc.tensor.matmul(ps[:, m, :], lhsT=whh(d, k, m),
                                         rhs=u_prev[:, k, :],
                                         start=False, stop=(k == K - 1),
                                         skip_group_check=True)

            # Warm every remaining psum buffer's has_written bits once with a
            # throwaway start=True group, so all real groups can use the
            # DVE-init + start=False accumulate path uniformly from step 0.
            # (pst/psp above already warmed one slot of ps1/ps2.)
            for pool, tag, n in ((ps0p, "ps0", 3), (ps1p, "ps1", 2),
                                 (ps2p, "ps2", 2)):
                for _ in range(n):
                    wtile = pool.tile([P, K, T], F32, tag=tag, name=f"warm_{tag}")
                    nc.tensor.matmul(wtile[:, :, :], lhsT=wih(0, 0, 0),
                                     rhs=zeros[:, :, :], start=True, stop=True)

            for t in range(S + 2):
                if t < S:
                    s = t
                    ps = ps0p.tile([P, K, T], F32, tag="ps0")
                    u = u0p.tile([P, K, T], BF16, tag="u0")
                    # pre0 column s broadcast over T via a 0-stride AP
                    src_ap, dst_ap = bass.broadcast_tensor_aps(
                        pre0_sb[:, :, s:s + 1], ps[:, :, :])
                    nc.vector.tensor_copy(dst_ap, src_ap)
                    rec_mms(ps, 0, None, u0[s - 1])
                    nc.scalar.activation(u[:, :, :], ps[:, :, :], TANH)
                    u0[s] = u
                if 1 <= t <= S:
                    s = t - 1
                    ps = ps1p.tile([P, K, T], F32, tag="ps1")
                    nc.vector.tensor_copy(ps[:, :, :], biasbc1)
                    rec_mms(ps, 1, u0[s], u1[s - 1])
                    u = u1p.tile([P, K, T], BF16, tag="u1")
                    nc.scalar.activation(u[:, :, :], ps[:, :, :], TANH)
                    u1[s] = u
                if 2 <= t:
                    s = t - 2
                    ps = ps2p.tile([P, K, T], F32, tag="ps2")
                    u2_prev = zeros if s == 0 else u2all[:, s - 1, :, :]
                    nc.vector.tensor_copy(ps[:, :, :], biasbc2)
                    rec_mms(ps, 2, u1[s], u2_prev)
                    nc.scalar.activation(u2all[:, s, :, :], ps[:, :, :], TANH)
                    if (s + 1) in OCHUNK_ENDS:
                        i = OCHUNK_ENDS.index(s + 1)
                        s0 = 0 if i == 0 else OCHUNK_ENDS[i - 1]
                        nc.gpsimd.dma_start(
                            out=out_c[:, s0:s + 1, :, :],
                            in_=u2all[:, s0:s + 1, :, :])
                for dd in (u0, u1):
                    dd.pop(t - 4, None)

    return nc


def _blob_h(seed, wT_ih, wT_hh):
    """Pack per-core bf16 constants into the [P, CWH] blob."""
    b = np.zeros((P, CWH), BF_NP)
    wih_packed = wT_ih.reshape(D, K, P, H).transpose(2, 0, 1, 3).reshape(
        P, D * K * H)
    whh_packed = wT_hh.reshape(D, K, P, H).transpose(2, 0, 1, 3).reshape(
        P, D * K * H)
    b[:, W0:W0 + K * H] = wih_packed[:, :K * H]          # depth 0 (early DMA)
    b[:, W1A:W1A + K * H] = whh_packed[:, :K * H]        # depth 0 (early DMA)
    b[:, W0B:W0B + (D - 1) * K * H] = wih_packed[:, K * H:]
    b[:, W1B:W1B + (D - 1) * K * H] = whh_packed[:, K * H:]
    b[:, S0:S0 + H] = seed
    b[:, I0:I0 + P] = np.eye(P, dtype=np.float32)
    return b


def _blob_f(bs):
    """Pack per-core fp32 bias columns: bsum[d, m*128+p] -> col B0 + d*K + m."""
    b = np.empty((P, CWF), np.float32)
    b[:, B0:B0 + D * K] = bs.reshape(D, K, P).transpose(2, 0, 1).reshape(P, D * K)
    return b


def kernel(src, trg, Wx_ih, Wx_hh, bx_ih, bx_hh, Wy_ih, Wy_hh, by_ih, by_hh):
    if "nc" not in _cache:
        _cache["nc"] = _build()
    nc = _cache["nc"]

    def tr(w):  # [D,H,H] -> W[d].T contiguous
        return np.ascontiguousarray(np.swapaxes(np.asarray(w, np.float32), 1, 2))

    src = np.asarray(src, np.float32)
    trg = np.asarray(trg, np.float32)
    wx_ihT, wx_hhT = tr(Wx_ih), tr(Wx_hh)
    wy_ihT, wy_hhT = tr(Wy_ih), tr(Wy_hh)
    bx = np.asarray(bx_ih, np.float32) + np.asarray(bx_hh, np.float32)
    by = np.asarray(by_ih, np.float32) + np.asarray(by_hh, np.float32)

    in_maps = []
    for b in range(B):  # cores 0-3: x chains
        in_maps.append({"blobh": _blob_h(src[b], wx_ihT, wx_hhT),
                        "blobf": _blob_f(bx)})
    for b in range(B):  # cores 4-7: y chains
        in_maps.append({"blobh": _blob_h(trg[b], wy_ihT, wy_hhT),
                        "blobf": _blob_f(by)})

    _cache["last_in_maps"] = in_maps
    globals()["_last_in_maps"] = in_maps
    res = run_bass_kernel_spmd(nc, in_maps, list(range(8)))

    out = np.empty((B, S, T, 2, H), np.float32)
    ii = np.arange(S)[:, None]
    jj = np.arange(T)[None, :]
    idx = (jj - ii) % T  # hx[i,j] = u_i[(j-i)%T]
    for b in range(B):
        # raw core output [p, s, k, v] -> [s, H=k*128+p, v]
        raw = np.asarray(res.results[b]["out"]).astype(np.float32)
        arr = raw.transpose(1, 2, 0, 3).reshape(S, H, T)
        hx = np.take_along_axis(arr, idx[:, None, :], axis=2)  # [s, H, j]
        out[b, :, :, 0, :] = hx.transpose(0, 2, 1)
        raw = np.asarray(res.results[B + b]["out"]).astype(np.float32)
        arr = raw.transpose(1, 2, 0, 3).reshape(S, H, T)
        out[b, :, :, 1, :] = arr.transpose(2, 0, 1)  # [j, H, i] -> [i, j, H]
    return out
